# revision 80
# baseline (speedup 1.0000x reference)
"""Multi-head attention (B=8, S=1024, D=1024, H=16) on 8 TRN2 NeuronCores.

Sharding: pure data-parallel over batch - each core computes one batch
element end-to-end (weights replicated per core), so no collectives.

Per-core pipeline (PE-bound design: ~177us of matmul rows at bf16, with
softmax exp on ScalarE (~133us) hidden underneath):
  1. Inputs DMA'd with f32->bf16 cast (SWDGE), ordered on the serial DMA
     FIFO so pair-0's projections start ~15us in; weights load into
     persistent tiles in per-2-pair column slices. A burst of junk
     matmuls at t~1us walks the PE p-state up to full clock before real
     work arrives.
  2. PE-transposes pack inputs into [P, kc, S] (contraction on
     partitions) via a scoped 2-bank PSUM pool; pair-0's QK projection
     matmuls slip underneath the DMA-paced transposes through the
     engines' 4-deep wait-queue bypass.
  3. Per head-pair p: QT/KT[p] = (x @ w)^T in f32 (f32r matmuls, bias
     folded into the PSUM drain); V natural orientation into VA[p]
     [j_part, jb, head, 65] (bf16) with a ones column.
  4. 16 score/exp slots per pair (head x j-block): scores^T[j,i] via 2
     f32r matmuls into a [P, S] PSUM tile (2 banks, bufs=3 ring); one
     exp(s/8 - 2.4) per slot on ScalarE straight out of PSUM -> bf16
     eT[p] [j_part, jb, head, i].
  5. PV natural: out[i, dk+1] accumulated over jb (lhsT = eT block,
     rhs = VA block); col 64 = sum(exp) via the ones column. Reciprocal
     + per-partition-scalar normalize into a per-pair staging tile, one
     [P, ib, 128] DMA per pair (split per head at the tail).
  6. Emission is software-pipelined: projections for pair p+1, PV of
     pair p (h0) and of pair p-1 (h1) ride in pair p's slots so the
     in-order PE queue never head-of-line blocks; pair 6's h0 PV and
     pair 7's V projection are deferred into projection-free pair 7.
     NOTE: a PSUM bank must only ever hold ONE open matmul accumulation
     group (start_tensor_calc zeroing is region-granular), so PV is
     strictly ib-major.
"""
import numpy as np
from contextlib import ExitStack

import concourse.bass as bass
import concourse.mybir as mybir
import concourse.tile as tile
from concourse import bacc
from concourse.bass_utils import run_bass_kernel_spmd
from concourse.masks import make_identity

F32 = mybir.dt.float32
F32R = mybir.dt.float32r
BF16 = mybir.dt.bfloat16
FP8 = mybir.dt.float8e4
DR = mybir.MatmulPerfMode.DoubleRow
EXP = mybir.ActivationFunctionType.Exp

B, S, D, H, DK = 8, 1024, 1024, 16, 64
P = 128
NB = S // P            # 8 row/col blocks
NPAIR = H // 2         # 8 head pairs
HALF = 512
N_CORES = 8
EXP_SCALE = 0.125      # 1/sqrt(dk)
EXP_BIAS = -2.4        # keeps bf16 exp well-scaled; fp8 would need <240

PV_FP8 = False         # probs/V in fp8e4 + DoubleRow PV (else bf16).
                       # fp8 measured 6e-2 rel err: attention rows here are
                       # concentrated, so 3.6% fp8 noise on dominant probs
                       # does not average out. bf16 keeps 7e-3.
_NO_DR = False         # debug: fp8 without DoubleRow perf mode
TAIL_JB_MAJOR = False  # pipeline the final head's PV across exp slots

_compiled = {}


def _build(use_bias=True, pv_fp8=PV_FP8):
    nc = bacc.Bacc("TRN2", target_bir_lowering=False, debug=False,
                   enable_asserts=False, num_devices=N_CORES)

    dq = nc.dram_tensor("q", [S, D], F32, kind="ExternalInput").ap()
    dk_ = nc.dram_tensor("k", [S, D], F32, kind="ExternalInput").ap()
    dv = nc.dram_tensor("v", [S, D], F32, kind="ExternalInput").ap()
    dwq = nc.dram_tensor("wq", [D, D], F32, kind="ExternalInput").ap()
    dwk = nc.dram_tensor("wk", [D, D], F32, kind="ExternalInput").ap()
    dwv = nc.dram_tensor("wv", [D, D], F32, kind="ExternalInput").ap()
    dbq = nc.dram_tensor("bq", [D], F32, kind="ExternalInput").ap()
    dbk = nc.dram_tensor("bk", [D], F32, kind="ExternalInput").ap()
    dbv = nc.dram_tensor("bv", [D], F32, kind="ExternalInput").ap()
    dout = nc.dram_tensor("out", [S, D], F32, kind="ExternalOutput").ap()

    PVDT = FP8 if pv_fp8 else BF16

    with tile.TileContext(nc) as tc:
        with ExitStack() as ctx:
            const = ctx.enter_context(tc.tile_pool(name="const", bufs=1))
            persist = ctx.enter_context(tc.tile_pool(name="persist", bufs=1))
            ring = ctx.enter_context(tc.tile_pool(name="ring", bufs=1))
            scr = ctx.enter_context(tc.tile_pool(name="scr", bufs=1,
                                                 space="PSUM"))

            ident_bf = const.tile([P, P], BF16)
            ones_bf = const.tile([1, P], BF16)
            ebias = const.tile([P, 1], F32)
            junk = const.tile([P, DK], BF16)

            qt = persist.tile([P, NB, S], BF16, name="qt")
            kt = persist.tile([P, NB, S], BF16, name="kt")
            vt = persist.tile([P, NB, S], BF16, name="vt")
            wqa = persist.tile([P, NB, D], BF16, name="wqa")
            wka = persist.tile([P, NB, D], BF16, name="wka")
            wva = persist.tile([P, NB, D], BF16, name="wva")
            bqt = persist.tile([P, NPAIR], F32, name="bqt")
            bkt = persist.tile([P, NPAIR], F32, name="bkt")
            bvr = persist.tile([1, D], BF16, name="bvr")

            # ---- DMA FIFO order tuned so pair-0's QK projections can
            # start as early as possible on the serial DMA device ----
            def load_nat(dsrc, nm, chunks):
                tiles = []
                for ci in chunks:
                    rr = ci * 2
                    natt = ring.tile([P, 2, S], BF16, name=f"nat_{nm}_{rr}",
                                     tag="nat", bufs=4)
                    nc.gpsimd.dma_start(
                        out=natt[:],
                        in_=dsrc[rr * P:(rr + 2) * P, :]
                            .rearrange("(r p) d -> p r d", p=P))
                    tiles.append(natt)
                return tiles

            def load_w_slice(wdst, wsrc, m):
                nc.gpsimd.dma_start(
                    out=wdst[:, :, m * 2 * P:(m + 1) * 2 * P],
                    in_=wsrc.rearrange("(c p) d -> p c d", p=P)
                            [:, :, m * 2 * P:(m + 1) * 2 * P])

            nc.vector.memset(junk[:], 0.5)   # first: unblocks PE warm-up
            natq = load_nat(dq, "q", (0,))
            # identity built right after the first input DMA is queued:
            # its Pool-engine ops must not queue behind all the SWDGE
            # generations, or the first transposes gate on the identity
            make_identity(nc, ident_bf)
            natq += load_nat(dq, "q", (1, 2, 3))
            nc.vector.memset(ones_bf[:], 1.0)
            nc.vector.memset(ebias[:], EXP_BIAS)
            load_w_slice(wqa, dwq, 0)
            natk = load_nat(dk_, "k", (0, 1, 2, 3))
            load_w_slice(wka, dwk, 0)
            natv = load_nat(dv, "v", (0, 1, 2, 3))
            load_w_slice(wva, dwv, 0)
            load_w_slice(wqa, dwq, 1)
            load_w_slice(wka, dwk, 1)
            load_w_slice(wva, dwv, 1)
            for m in range(2, 4):
                load_w_slice(wqa, dwq, m)
                load_w_slice(wka, dwk, m)
                load_w_slice(wva, dwv, m)
            if use_bias:
                nc.sync.dma_start(bqt[:], dbq.rearrange("(c p) -> p c", p=P))
                nc.sync.dma_start(bkt[:], dbk.rearrange("(c p) -> p c", p=P))
                nc.gpsimd.dma_start(out=bvr[:], in_=dbv[None, :])

            # ---- PE warm-up: junk matmuls walk the p-state up while the
            # first input DMAs land ----
            with tc.tile_pool(name="warm_ps", bufs=1, space="PSUM") as wps:
                for i in range(40):
                    wt = wps.tile([P, DK], F32, name=f"wpsum{i}", tag="warm",
                                  bufs=2)
                    nc.tensor.matmul(wt[0:DK, :], junk[:], junk[:],
                                     start=True, stop=True)

            sc = None  # created after the transpose pool is released

            def tr_chunk(dst, natt, rr, nm, pool):
                for r2 in range(2):
                    tpt = pool.tile([P, NB, P], BF16,
                                    name=f"tpt_{nm}_{rr}_{r2}",
                                    tag="tp", bufs=2)
                    for c in range(NB):
                        nc.tensor.transpose(
                            tpt[:, c, :], natt[:, r2, c * P:(c + 1) * P],
                            ident_bf[:])
                    nc.vector.tensor_copy(
                        dst[:, :, (rr + r2) * P:(rr + r2 + 1) * P], tpt[:])

            cur = {}   # live per-pair tiles: (kind, p) -> AP

            def qk_unit(p, tens, hf):
                """QT/KT[p][:, hf] = ((x @ w)^T + b) half, f32."""
                key = ("QT" if tens == 0 else "KT", p)
                if key not in cur:
                    cur[key] = ring.tile([P, S], F32R, name=f"{key[0]}{p}",
                                         tag=key[0], bufs=2)
                dst = cur[key]
                wsrc = wqa if tens == 0 else wka
                bsrc = bqt if tens == 0 else bkt
                xsrc = qt if tens == 0 else kt
                pj = scr.tile([P, HALF], F32, name=f"pj_{p}_{tens}_{hf}",
                              tag="scr", bufs=2)
                for kc in range(NB):
                    nc.tensor.matmul(pj[:], wsrc[:, kc, p * P:(p + 1) * P],
                                     xsrc[:, kc, hf * HALF:(hf + 1) * HALF],
                                     start=(kc == 0), stop=(kc == NB - 1))
                if use_bias:
                    nc.vector.tensor_scalar_add(
                        dst[:, hf * HALF:(hf + 1) * HALF], pj[:],
                        bsrc[:, p:p + 1])
                else:
                    nc.vector.tensor_copy(
                        dst[:, hf * HALF:(hf + 1) * HALF], pj[:])

            def v_unit(p, half):
                """VA[p][:, 4 j-blocks, head, 0:64] = (v @ w_v + b_v)."""
                key = ("VA", p)
                if key not in cur:
                    cur[key] = ring.tile([P, NB, 2, DK + 1], PVDT,
                                         name=f"VA{p}", tag="VA", bufs=2)
                    nc.vector.memset(cur[key][:, :, :, DK:DK + 1], 1.0)
                va = cur[key]
                pjv = scr.tile([P, HALF], F32, name=f"pjv_{p}_{half}",
                               tag="scr", bufs=2)
                pv4 = pjv.rearrange("q (r f) -> q r f", f=P)
                for r4 in range(4):
                    r = half * 4 + r4
                    for kc in range(NB):
                        nc.tensor.matmul(
                            pv4[:, r4, :], vt[:, kc, r * P:(r + 1) * P],
                            wva[:, kc, p * P:(p + 1) * P],
                            start=(kc == 0),
                            stop=(not use_bias and kc == NB - 1))
                    if use_bias:
                        nc.tensor.matmul(pv4[:, r4, :], ones_bf[:],
                                         bvr[:, p * P:(p + 1) * P],
                                         start=False, stop=True)
                nc.vector.tensor_copy(
                    va[:, half * 4:(half + 1) * 4, :, 0:DK],
                    pjv.rearrange("q (r hh f) -> q r hh f", r=4, hh=2))

            def score_slot(p, hh, jb):
                """scores^T for one (head, j-block) -> exp -> eT."""
                QTp, KTp = cur[("QT", p)], cur[("KT", p)]
                eTp = cur[("eT", p)]
                sct = sc.tile([P, S], F32, name=f"sc_{p}_{hh}_{jb}",
                              tag="sc", bufs=3)
                for ih in range(2):
                    nc.tensor.matmul(
                        sct[:, ih * HALF:(ih + 1) * HALF],
                        KTp[hh * DK:(hh + 1) * DK, jb * P:(jb + 1) * P],
                        QTp[hh * DK:(hh + 1) * DK,
                            ih * HALF:(ih + 1) * HALF],
                        start=True, stop=True)
                nc.scalar.activation(
                    out=eTp[:, jb, hh, :], in_=sct[:],
                    func=EXP, scale=EXP_SCALE, bias=ebias[:])

            def pv_mms(pu4, eTp, VAp, hh, half, jbs, start, stop):
                # ib-major: a PSUM bank must hold only ONE open accumulation
                # group at a time (start_tensor_calc zeroing is bank-granular)
                for i4 in range(4):
                    ib = half * 4 + i4
                    for jb in jbs:
                        nc.tensor.matmul(
                            pu4[:, i4, 0:DK + 1],
                            eTp[:, jb, hh, ib * P:(ib + 1) * P],
                            VAp[:, jb, hh, :],
                            start=(jb == jbs[0] and start),
                            stop=(jb == jbs[-1] and stop))

            def pv_norm(pu4, stage, hh, half):
                rcpt = ring.tile([P, 4], F32, name=f"rcp_{hh}_{half}",
                                 tag="rcp", bufs=4)
                nc.vector.reciprocal(rcpt[:], pu4[:, :, DK])
                for i4 in range(4):
                    nc.vector.tensor_scalar_mul(
                        stage[:, half * 4 + i4, hh * DK:(hh + 1) * DK],
                        pu4[:, i4, 0:DK], rcpt[:, i4:i4 + 1])

            def pv_unit(p, hh, half, stage):
                """out[i, 0:65] for 4 i-blocks; normalize into stage."""
                eTp, VAp = cur[("eT", p)], cur[("VA", p)]
                pu = scr.tile([P, HALF], F32, name=f"pu_{p}_{hh}_{half}",
                              tag="scr", bufs=2)
                pu4 = pu.rearrange("q (i f) -> q i f", f=P)
                if pv_fp8 and not _NO_DR:
                    for i4 in range(4):
                        for t in range(4):
                            nc.tensor.matmul(
                                pu4[:, i4, 0:DK + 1],
                                eTp[:, 2 * t:2 * t + 2, hh,
                                    (half * 4 + i4) * P:(half * 4 + i4 + 1) * P],
                                VAp[:, 2 * t:2 * t + 2, hh, :],
                                start=(t == 0), stop=(t == 3), perf_mode=DR)
                else:
                    pv_mms(pu4, eTp, VAp, hh, half, list(range(NB)),
                           True, True)
                pv_norm(pu4, stage, hh, half)

            def out_dma(p, stage):
                nc.sync.dma_start(
                    dout[:, p * P:(p + 1) * P]
                    .rearrange("(ib q) c -> q ib c", q=P), stage[:])

            def out_dma_part(p, stage, hh, half, engine):
                engine.dma_start(
                    dout[half * HALF:(half + 1) * HALF,
                         p * P + hh * DK:p * P + (hh + 1) * DK]
                    .rearrange("(ib q) c -> q ib c", q=P),
                    stage[:, half * 4:(half + 1) * 4, hh * DK:(hh + 1) * DK])

            # ---- startup: all transposes (scoped PSUM pool), then pair-0
            # QK projections; the engines' wait-queue bypass interleaves
            # the projection matmuls under the DMA-paced transposes ----
            with tc.tile_pool(name="tp_ps", bufs=1, space="PSUM") as tps:
                for rr in range(0, NB, 2):
                    tr_chunk(qt, natq[rr // 2], rr, "q", tps)
                qk_unit(0, 0, 0)
                qk_unit(0, 0, 1)
                for rr in range(0, NB, 2):
                    tr_chunk(kt, natk[rr // 2], rr, "k", tps)
                qk_unit(0, 1, 0)
                qk_unit(0, 1, 1)
                for rr in range(0, NB, 2):
                    tr_chunk(vt, natv[rr // 2], rr, "v", tps)

            sc = ctx.enter_context(tc.tile_pool(name="sc", bufs=1,
                                                space="PSUM"))

            stages = {}
            for p in range(NPAIR):
                cur[("eT", p)] = ring.tile([P, NB, 2, S], PVDT,
                                           name=f"eT{p}", tag="eT",
                                           bufs=2)
                stages[p] = ring.tile([P, NB, P], F32, name=f"stage{p}",
                                      tag="stage", bufs=2)
                nxt = p + 1 if p + 1 < NPAIR else None
                fillers = {}
                if p == 0:
                    fillers[7] = lambda: v_unit(0, 0)
                    fillers[9] = lambda: v_unit(0, 1)
                else:
                    fillers[0] = lambda: pv_unit(p - 1, 1, 0, stages[p - 1])
                    def _f1():
                        pv_unit(p - 1, 1, 1, stages[p - 1])
                        if p - 1 != NPAIR - 2:
                            out_dma(p - 1, stages[p - 1])
                    fillers[1] = _f1
                    if p == NPAIR - 1:
                        fillers[2] = lambda: pv_unit(p - 1, 0, 0,
                                                     stages[p - 1])
                        def _f3():
                            pv_unit(p - 1, 0, 1, stages[p - 1])
                            out_dma(p - 1, stages[p - 1])
                        fillers[3] = _f3
                        fillers[5] = lambda: v_unit(p, 1)
                if nxt is not None:
                    qslots = (5, 6, 8, 10) if p == 0 else (2, 4, 6, 8)
                    fillers[qslots[0]] = lambda: qk_unit(nxt, 0, 0)
                    fillers[qslots[1]] = lambda: qk_unit(nxt, 0, 1)
                    fillers[qslots[2]] = lambda: qk_unit(nxt, 1, 0)
                    fillers[qslots[3]] = lambda: qk_unit(nxt, 1, 1)
                    fillers[11 if p == 0 else 10] = lambda: v_unit(nxt, 0)
                    if nxt != NPAIR - 1:
                        fillers[14] = lambda: v_unit(nxt, 1)
                if p != NPAIR - 2:
                    fillers[12] = lambda: pv_unit(p, 0, 0, stages[p])
                    def _f13():
                        pv_unit(p, 0, 1, stages[p])
                        if p == NPAIR - 1:
                            out_dma_part(p, stages[p], 0, 0, nc.sync)
                            out_dma_part(p, stages[p], 0, 1, nc.sync)
                    fillers[13] = _f13

                tail_pus = None
                si = 0
                for hh in range(2):
                    for jb in range(NB):
                        score_slot(p, hh, jb)
                        if si in fillers:
                            fillers[si]()
                        if p == NPAIR - 1 and si == 14 and TAIL_JB_MAJOR:
                            # final head's PV, jb-major: everything except
                            # jb=7 can run before the last exp lands
                            eTp, VAp = cur[("eT", p)], cur[("VA", p)]
                            tail_pus = []
                            for half in range(2):
                                pu = scr.tile([P, HALF], F32,
                                              name=f"pu_t_{half}",
                                              tag="scr", bufs=2)
                                tail_pus.append(
                                    pu.rearrange("q (i f) -> q i f", f=P))
                            for half in range(2):
                                pv_mms(tail_pus[half], eTp, VAp, 1, half,
                                       list(range(NB - 1)), True, False)
                        si += 1
                for k in (("QT", p), ("KT", p)):
                    cur.pop(k, None)

            # tail: last jb of the final head's PV, normalize, DMA out
            eTp, VAp = cur[("eT", NPAIR - 1)], cur[("VA", NPAIR - 1)]
            stage = stages[NPAIR - 1]
            if TAIL_JB_MAJOR:
                for half in range(2):
                    pv_mms(tail_pus[half], eTp, VAp, 1, half, [NB - 1],
                           False, True)
                for half in range(2):
                    pv_norm(tail_pus[half], stage, 1, half)
                    out_dma_part(NPAIR - 1, stage, 1, half, nc.scalar)
            else:
                for half in range(2):
                    pv_unit(NPAIR - 1, 1, half, stage)
                    # two different queues so the final DMAs don't serialize
                    out_dma_part(NPAIR - 1, stage, 1, half,
                                 nc.scalar if half == 0 else nc.sync)

    nc.compile()
    return nc


def kernel(q, k, v, w_q, b_q, w_k, b_k, w_v, b_v):
    use_bias = bool(np.any(np.asarray(b_q)) or np.any(np.asarray(b_k))
                    or np.any(np.asarray(b_v)))
    key = (use_bias, PV_FP8, _NO_DR)
    if key not in _compiled:
        _compiled[key] = _build(use_bias, PV_FP8)
    nc = _compiled[key]

    f = lambda x: np.ascontiguousarray(np.asarray(x, dtype=np.float32))
    in_maps = []
    for c in range(N_CORES):
        in_maps.append({
            "q": f(q[c]), "k": f(k[c]), "v": f(v[c]),
            "wq": f(w_q), "wk": f(w_k), "wv": f(w_v),
            "bq": f(b_q), "bk": f(b_k), "bv": f(b_v),
        })
    res = run_bass_kernel_spmd(nc, in_maps, list(range(N_CORES)))
    out = np.stack([res.results[c]["out"] for c in range(N_CORES)], axis=0)
    kernel.last_results = res
    return out


# revision 81
# speedup vs baseline: 1.0024x; 1.0024x over previous
"""Multi-head attention (B=8, S=1024, D=1024, H=16) on 8 TRN2 NeuronCores.

Sharding: pure data-parallel over batch - each core computes one batch
element end-to-end (weights replicated per core), so no collectives.

Per-core pipeline (PE-bound design: ~177us of matmul rows at bf16, with
softmax exp on ScalarE (~133us) hidden underneath):
  1. Inputs DMA'd with f32->bf16 cast (SWDGE), ordered on the serial DMA
     FIFO so pair-0's projections start ~15us in; weights load into
     persistent tiles in per-2-pair column slices. A burst of junk
     matmuls at t~1us walks the PE p-state up to full clock before real
     work arrives.
  2. PE-transposes pack inputs into [P, kc, S] (contraction on
     partitions) via a scoped 2-bank PSUM pool; pair-0's QK projection
     matmuls slip underneath the DMA-paced transposes through the
     engines' 4-deep wait-queue bypass.
  3. Per head-pair p: QT/KT[p] = (x @ w)^T in f32 (f32r matmuls, bias
     folded into the PSUM drain); V natural orientation into VA[p]
     [j_part, jb, head, 65] (bf16) with a ones column.
  4. 16 score/exp slots per pair (head x j-block): scores^T[j,i] via 2
     f32r matmuls into a [P, S] PSUM tile (2 banks, bufs=3 ring); one
     exp(s/8 - 2.4) per slot on ScalarE straight out of PSUM -> bf16
     eT[p] [j_part, jb, head, i].
  5. PV natural: out[i, dk+1] accumulated over jb (lhsT = eT block,
     rhs = VA block); col 64 = sum(exp) via the ones column. Reciprocal
     + per-partition-scalar normalize into a per-pair staging tile, one
     [P, ib, 128] DMA per pair (split per head at the tail).
  6. Emission is software-pipelined: projections for pair p+1, PV of
     pair p (h0) and of pair p-1 (h1) ride in pair p's slots so the
     in-order PE queue never head-of-line blocks; pair 6's h0 PV and
     pair 7's V projection are deferred into projection-free pair 7.
     NOTE: a PSUM bank must only ever hold ONE open matmul accumulation
     group (start_tensor_calc zeroing is region-granular), so PV is
     strictly ib-major.
"""
import numpy as np
from contextlib import ExitStack

import concourse.bass as bass
import concourse.mybir as mybir
import concourse.tile as tile
from concourse import bacc
from concourse.bass_utils import run_bass_kernel_spmd
from concourse.masks import make_identity

F32 = mybir.dt.float32
F32R = mybir.dt.float32r
BF16 = mybir.dt.bfloat16
FP8 = mybir.dt.float8e4
DR = mybir.MatmulPerfMode.DoubleRow
EXP = mybir.ActivationFunctionType.Exp

B, S, D, H, DK = 8, 1024, 1024, 16, 64
P = 128
NB = S // P            # 8 row/col blocks
NPAIR = H // 2         # 8 head pairs
HALF = 512
N_CORES = 8
EXP_SCALE = 0.125      # 1/sqrt(dk)
EXP_BIAS = -2.4        # keeps bf16 exp well-scaled; fp8 would need <240

PV_FP8 = False         # probs/V in fp8e4 + DoubleRow PV (else bf16).
                       # fp8 measured 6e-2 rel err: attention rows here are
                       # concentrated, so 3.6% fp8 noise on dominant probs
                       # does not average out. bf16 keeps 7e-3.
_NO_DR = False         # debug: fp8 without DoubleRow perf mode
TAIL_JB_MAJOR = False  # pipeline the final head's PV across exp slots

_compiled = {}


def _build(use_bias=True, pv_fp8=PV_FP8):
    nc = bacc.Bacc("TRN2", target_bir_lowering=False, debug=False,
                   enable_asserts=False, num_devices=N_CORES)

    dq = nc.dram_tensor("q", [S, D], F32, kind="ExternalInput").ap()
    dk_ = nc.dram_tensor("k", [S, D], F32, kind="ExternalInput").ap()
    dv = nc.dram_tensor("v", [S, D], F32, kind="ExternalInput").ap()
    dwq = nc.dram_tensor("wq", [D, D], F32, kind="ExternalInput").ap()
    dwk = nc.dram_tensor("wk", [D, D], F32, kind="ExternalInput").ap()
    dwv = nc.dram_tensor("wv", [D, D], F32, kind="ExternalInput").ap()
    dbq = nc.dram_tensor("bq", [D], F32, kind="ExternalInput").ap()
    dbk = nc.dram_tensor("bk", [D], F32, kind="ExternalInput").ap()
    dbv = nc.dram_tensor("bv", [D], F32, kind="ExternalInput").ap()
    dout = nc.dram_tensor("out", [S, D], F32, kind="ExternalOutput").ap()

    PVDT = FP8 if pv_fp8 else BF16

    with tile.TileContext(nc) as tc:
        with ExitStack() as ctx:
            const = ctx.enter_context(tc.tile_pool(name="const", bufs=1))
            persist = ctx.enter_context(tc.tile_pool(name="persist", bufs=1))
            ring = ctx.enter_context(tc.tile_pool(name="ring", bufs=1))
            scr = ctx.enter_context(tc.tile_pool(name="scr", bufs=1,
                                                 space="PSUM"))

            ident_bf = const.tile([P, P], BF16)
            ones_bf = const.tile([1, P], BF16)
            ebias = const.tile([P, 1], F32)
            junk = const.tile([P, DK], BF16)

            qt = persist.tile([P, NB, S], BF16, name="qt")
            kt = persist.tile([P, NB, S], BF16, name="kt")
            vt = persist.tile([P, NB, S], BF16, name="vt")
            wqa = persist.tile([P, NB, D], BF16, name="wqa")
            wka = persist.tile([P, NB, D], BF16, name="wka")
            wva = persist.tile([P, NB, D], BF16, name="wva")
            bqt = persist.tile([P, NPAIR], F32, name="bqt")
            bkt = persist.tile([P, NPAIR], F32, name="bkt")
            bvr = persist.tile([1, D], BF16, name="bvr")

            # ---- DMA FIFO order tuned so pair-0's QK projections can
            # start as early as possible on the serial DMA device ----
            def load_nat(dsrc, nm, chunks):
                tiles = []
                for ci in chunks:
                    rr = ci * 2
                    natt = ring.tile([P, 2, S], BF16, name=f"nat_{nm}_{rr}",
                                     tag="nat", bufs=4)
                    nc.gpsimd.dma_start(
                        out=natt[:],
                        in_=dsrc[rr * P:(rr + 2) * P, :]
                            .rearrange("(r p) d -> p r d", p=P))
                    tiles.append(natt)
                return tiles

            def load_w_slice(wdst, wsrc, m):
                nc.gpsimd.dma_start(
                    out=wdst[:, :, m * 2 * P:(m + 1) * 2 * P],
                    in_=wsrc.rearrange("(c p) d -> p c d", p=P)
                            [:, :, m * 2 * P:(m + 1) * 2 * P])

            nc.vector.memset(junk[:], 0.5)   # first: unblocks PE warm-up
            natq = load_nat(dq, "q", (0,))
            # identity built right after the first input DMA is queued:
            # its Pool-engine ops must not queue behind all the SWDGE
            # generations, or the first transposes gate on the identity
            make_identity(nc, ident_bf)
            natq += load_nat(dq, "q", (1, 2, 3))
            nc.vector.memset(ones_bf[:], 1.0)
            nc.vector.memset(ebias[:], EXP_BIAS)
            load_w_slice(wqa, dwq, 0)
            natk = load_nat(dk_, "k", (0, 1, 2, 3))
            load_w_slice(wka, dwk, 0)
            natv = load_nat(dv, "v", (0, 1, 2, 3))
            load_w_slice(wva, dwv, 0)
            load_w_slice(wqa, dwq, 1)
            load_w_slice(wka, dwk, 1)
            load_w_slice(wva, dwv, 1)
            for m in range(2, 4):
                load_w_slice(wqa, dwq, m)
                load_w_slice(wka, dwk, m)
                load_w_slice(wva, dwv, m)
            if use_bias:
                nc.sync.dma_start(bqt[:], dbq.rearrange("(c p) -> p c", p=P))
                nc.sync.dma_start(bkt[:], dbk.rearrange("(c p) -> p c", p=P))
                nc.gpsimd.dma_start(out=bvr[:], in_=dbv[None, :])

            # ---- PE warm-up: junk matmuls walk the p-state up while the
            # first input DMAs land ----
            with tc.tile_pool(name="warm_ps", bufs=1, space="PSUM") as wps:
                for i in range(28):
                    wt = wps.tile([P, DK], F32, name=f"wpsum{i}", tag="warm",
                                  bufs=2)
                    nc.tensor.matmul(wt[0:DK, :], junk[:], junk[:],
                                     start=True, stop=True)

            sc = None  # created after the transpose pool is released

            def tr_chunk(dst, natt, rr, nm, pool):
                for r2 in range(2):
                    tpt = pool.tile([P, NB, P], BF16,
                                    name=f"tpt_{nm}_{rr}_{r2}",
                                    tag="tp", bufs=2)
                    for c in range(NB):
                        nc.tensor.transpose(
                            tpt[:, c, :], natt[:, r2, c * P:(c + 1) * P],
                            ident_bf[:])
                    nc.vector.tensor_copy(
                        dst[:, :, (rr + r2) * P:(rr + r2 + 1) * P], tpt[:])

            cur = {}   # live per-pair tiles: (kind, p) -> AP

            def qk_unit(p, tens, hf):
                """QT/KT[p][:, hf] = ((x @ w)^T + b) half, f32."""
                key = ("QT" if tens == 0 else "KT", p)
                if key not in cur:
                    cur[key] = ring.tile([P, S], F32R, name=f"{key[0]}{p}",
                                         tag=key[0], bufs=2)
                dst = cur[key]
                wsrc = wqa if tens == 0 else wka
                bsrc = bqt if tens == 0 else bkt
                xsrc = qt if tens == 0 else kt
                pj = scr.tile([P, HALF], F32, name=f"pj_{p}_{tens}_{hf}",
                              tag="scr", bufs=2)
                for kc in range(NB):
                    nc.tensor.matmul(pj[:], wsrc[:, kc, p * P:(p + 1) * P],
                                     xsrc[:, kc, hf * HALF:(hf + 1) * HALF],
                                     start=(kc == 0), stop=(kc == NB - 1))
                if use_bias:
                    nc.vector.tensor_scalar_add(
                        dst[:, hf * HALF:(hf + 1) * HALF], pj[:],
                        bsrc[:, p:p + 1])
                else:
                    nc.vector.tensor_copy(
                        dst[:, hf * HALF:(hf + 1) * HALF], pj[:])

            def v_unit(p, half):
                """VA[p][:, 4 j-blocks, head, 0:64] = (v @ w_v + b_v)."""
                key = ("VA", p)
                if key not in cur:
                    cur[key] = ring.tile([P, NB, 2, DK + 1], PVDT,
                                         name=f"VA{p}", tag="VA", bufs=2)
                    nc.vector.memset(cur[key][:, :, :, DK:DK + 1], 1.0)
                va = cur[key]
                pjv = scr.tile([P, HALF], F32, name=f"pjv_{p}_{half}",
                               tag="scr", bufs=2)
                pv4 = pjv.rearrange("q (r f) -> q r f", f=P)
                for r4 in range(4):
                    r = half * 4 + r4
                    for kc in range(NB):
                        nc.tensor.matmul(
                            pv4[:, r4, :], vt[:, kc, r * P:(r + 1) * P],
                            wva[:, kc, p * P:(p + 1) * P],
                            start=(kc == 0),
                            stop=(not use_bias and kc == NB - 1))
                    if use_bias:
                        nc.tensor.matmul(pv4[:, r4, :], ones_bf[:],
                                         bvr[:, p * P:(p + 1) * P],
                                         start=False, stop=True)
                nc.vector.tensor_copy(
                    va[:, half * 4:(half + 1) * 4, :, 0:DK],
                    pjv.rearrange("q (r hh f) -> q r hh f", r=4, hh=2))

            def score_slot(p, hh, jb):
                """scores^T for one (head, j-block) -> exp -> eT."""
                QTp, KTp = cur[("QT", p)], cur[("KT", p)]
                eTp = cur[("eT", p)]
                sct = sc.tile([P, S], F32, name=f"sc_{p}_{hh}_{jb}",
                              tag="sc", bufs=3)
                for ih in range(2):
                    nc.tensor.matmul(
                        sct[:, ih * HALF:(ih + 1) * HALF],
                        KTp[hh * DK:(hh + 1) * DK, jb * P:(jb + 1) * P],
                        QTp[hh * DK:(hh + 1) * DK,
                            ih * HALF:(ih + 1) * HALF],
                        start=True, stop=True)
                nc.scalar.activation(
                    out=eTp[:, jb, hh, :], in_=sct[:],
                    func=EXP, scale=EXP_SCALE, bias=ebias[:])

            def pv_mms(pu4, eTp, VAp, hh, half, jbs, start, stop):
                # ib-major: a PSUM bank must hold only ONE open accumulation
                # group at a time (start_tensor_calc zeroing is bank-granular)
                for i4 in range(4):
                    ib = half * 4 + i4
                    for jb in jbs:
                        nc.tensor.matmul(
                            pu4[:, i4, 0:DK + 1],
                            eTp[:, jb, hh, ib * P:(ib + 1) * P],
                            VAp[:, jb, hh, :],
                            start=(jb == jbs[0] and start),
                            stop=(jb == jbs[-1] and stop))

            def pv_norm(pu4, stage, hh, half):
                rcpt = ring.tile([P, 4], F32, name=f"rcp_{hh}_{half}",
                                 tag="rcp", bufs=4)
                nc.vector.reciprocal(rcpt[:], pu4[:, :, DK])
                for i4 in range(4):
                    nc.vector.tensor_scalar_mul(
                        stage[:, half * 4 + i4, hh * DK:(hh + 1) * DK],
                        pu4[:, i4, 0:DK], rcpt[:, i4:i4 + 1])

            def pv_unit(p, hh, half, stage):
                """out[i, 0:65] for 4 i-blocks; normalize into stage."""
                eTp, VAp = cur[("eT", p)], cur[("VA", p)]
                pu = scr.tile([P, HALF], F32, name=f"pu_{p}_{hh}_{half}",
                              tag="scr", bufs=2)
                pu4 = pu.rearrange("q (i f) -> q i f", f=P)
                if pv_fp8 and not _NO_DR:
                    for i4 in range(4):
                        for t in range(4):
                            nc.tensor.matmul(
                                pu4[:, i4, 0:DK + 1],
                                eTp[:, 2 * t:2 * t + 2, hh,
                                    (half * 4 + i4) * P:(half * 4 + i4 + 1) * P],
                                VAp[:, 2 * t:2 * t + 2, hh, :],
                                start=(t == 0), stop=(t == 3), perf_mode=DR)
                else:
                    pv_mms(pu4, eTp, VAp, hh, half, list(range(NB)),
                           True, True)
                pv_norm(pu4, stage, hh, half)

            def out_dma(p, stage):
                nc.sync.dma_start(
                    dout[:, p * P:(p + 1) * P]
                    .rearrange("(ib q) c -> q ib c", q=P), stage[:])

            def out_dma_part(p, stage, hh, half, engine):
                engine.dma_start(
                    dout[half * HALF:(half + 1) * HALF,
                         p * P + hh * DK:p * P + (hh + 1) * DK]
                    .rearrange("(ib q) c -> q ib c", q=P),
                    stage[:, half * 4:(half + 1) * 4, hh * DK:(hh + 1) * DK])

            # ---- startup: all transposes (scoped PSUM pool), then pair-0
            # QK projections; the engines' wait-queue bypass interleaves
            # the projection matmuls under the DMA-paced transposes ----
            with tc.tile_pool(name="tp_ps", bufs=1, space="PSUM") as tps:
                for rr in range(0, NB, 2):
                    tr_chunk(qt, natq[rr // 2], rr, "q", tps)
                qk_unit(0, 0, 0)
                qk_unit(0, 0, 1)
                for rr in range(0, NB, 2):
                    tr_chunk(kt, natk[rr // 2], rr, "k", tps)
                qk_unit(0, 1, 0)
                qk_unit(0, 1, 1)
                for rr in range(0, NB, 2):
                    tr_chunk(vt, natv[rr // 2], rr, "v", tps)

            sc = ctx.enter_context(tc.tile_pool(name="sc", bufs=1,
                                                space="PSUM"))

            stages = {}
            for p in range(NPAIR):
                cur[("eT", p)] = ring.tile([P, NB, 2, S], PVDT,
                                           name=f"eT{p}", tag="eT",
                                           bufs=2)
                stages[p] = ring.tile([P, NB, P], F32, name=f"stage{p}",
                                      tag="stage", bufs=2)
                nxt = p + 1 if p + 1 < NPAIR else None
                fillers = {}
                if p == 0:
                    fillers[7] = lambda: v_unit(0, 0)
                    fillers[9] = lambda: v_unit(0, 1)
                else:
                    fillers[0] = lambda: pv_unit(p - 1, 1, 0, stages[p - 1])
                    def _f1():
                        pv_unit(p - 1, 1, 1, stages[p - 1])
                        if p - 1 != NPAIR - 2:
                            out_dma(p - 1, stages[p - 1])
                    fillers[1] = _f1
                    if p == NPAIR - 1:
                        fillers[2] = lambda: pv_unit(p - 1, 0, 0,
                                                     stages[p - 1])
                        def _f3():
                            pv_unit(p - 1, 0, 1, stages[p - 1])
                            out_dma(p - 1, stages[p - 1])
                        fillers[3] = _f3
                        fillers[5] = lambda: v_unit(p, 1)
                if nxt is not None:
                    qslots = (5, 6, 8, 10) if p == 0 else (2, 4, 6, 8)
                    fillers[qslots[0]] = lambda: qk_unit(nxt, 0, 0)
                    fillers[qslots[1]] = lambda: qk_unit(nxt, 0, 1)
                    fillers[qslots[2]] = lambda: qk_unit(nxt, 1, 0)
                    fillers[qslots[3]] = lambda: qk_unit(nxt, 1, 1)
                    fillers[11 if p == 0 else 10] = lambda: v_unit(nxt, 0)
                    if nxt != NPAIR - 1:
                        fillers[14] = lambda: v_unit(nxt, 1)
                if p != NPAIR - 2:
                    fillers[12] = lambda: pv_unit(p, 0, 0, stages[p])
                    def _f13():
                        pv_unit(p, 0, 1, stages[p])
                        if p == NPAIR - 1:
                            out_dma_part(p, stages[p], 0, 0, nc.sync)
                            out_dma_part(p, stages[p], 0, 1, nc.sync)
                    fillers[13] = _f13

                tail_pus = None
                si = 0
                for hh in range(2):
                    for jb in range(NB):
                        score_slot(p, hh, jb)
                        if si in fillers:
                            fillers[si]()
                        if p == NPAIR - 1 and si == 14 and TAIL_JB_MAJOR:
                            # final head's PV, jb-major: everything except
                            # jb=7 can run before the last exp lands
                            eTp, VAp = cur[("eT", p)], cur[("VA", p)]
                            tail_pus = []
                            for half in range(2):
                                pu = scr.tile([P, HALF], F32,
                                              name=f"pu_t_{half}",
                                              tag="scr", bufs=2)
                                tail_pus.append(
                                    pu.rearrange("q (i f) -> q i f", f=P))
                            for half in range(2):
                                pv_mms(tail_pus[half], eTp, VAp, 1, half,
                                       list(range(NB - 1)), True, False)
                        si += 1
                for k in (("QT", p), ("KT", p)):
                    cur.pop(k, None)

            # tail: last jb of the final head's PV, normalize, DMA out
            eTp, VAp = cur[("eT", NPAIR - 1)], cur[("VA", NPAIR - 1)]
            stage = stages[NPAIR - 1]
            if TAIL_JB_MAJOR:
                for half in range(2):
                    pv_mms(tail_pus[half], eTp, VAp, 1, half, [NB - 1],
                           False, True)
                for half in range(2):
                    pv_norm(tail_pus[half], stage, 1, half)
                    out_dma_part(NPAIR - 1, stage, 1, half, nc.scalar)
            else:
                for half in range(2):
                    pv_unit(NPAIR - 1, 1, half, stage)
                    # two different queues so the final DMAs don't serialize
                    out_dma_part(NPAIR - 1, stage, 1, half,
                                 nc.scalar if half == 0 else nc.sync)

    nc.compile()
    return nc


def kernel(q, k, v, w_q, b_q, w_k, b_k, w_v, b_v):
    use_bias = bool(np.any(np.asarray(b_q)) or np.any(np.asarray(b_k))
                    or np.any(np.asarray(b_v)))
    key = (use_bias, PV_FP8, _NO_DR)
    if key not in _compiled:
        _compiled[key] = _build(use_bias, PV_FP8)
    nc = _compiled[key]

    f = lambda x: np.ascontiguousarray(np.asarray(x, dtype=np.float32))
    in_maps = []
    for c in range(N_CORES):
        in_maps.append({
            "q": f(q[c]), "k": f(k[c]), "v": f(v[c]),
            "wq": f(w_q), "wk": f(w_k), "wv": f(w_v),
            "bq": f(b_q), "bk": f(b_k), "bv": f(b_v),
        })
    res = run_bass_kernel_spmd(nc, in_maps, list(range(N_CORES)))
    out = np.stack([res.results[c]["out"] for c in range(N_CORES)], axis=0)
    kernel.last_results = res
    return out
